# revision 7
# baseline (speedup 1.0000x reference)
"""Trainium2 Bass kernel for nn_Distiller (attention-transfer distillation loss).

Computes on 8 NeuronCores (data-parallel over batch, 2 batches/core):
  SA part: per batch, weighted spatial grams A = V^T V with V = F * sqrt(Fc)
           (Fc = sum |F| over space), for teacher/student features; then
           rho_m = <A_S[m], A_T[m]> / (|A_S[m]| |A_T[m]|) per row.
           Uses the identity sum_n (Ahat_S - Ahat_T)^2 = 2 - 2*rho per row.
  IC part: per batch, channel grams G = L L^T of the [21, 16384] logit maps,
           same rho identity on the 21 rows of G.
Device emits per-row rho partial sums; host assembles the two scalar losses.
s_out passes through on host.
"""

import sys

if "/opt/trn_rl_repo" not in sys.path:
    sys.path.insert(0, "/opt/trn_rl_repo")

import numpy as np

import concourse.bass as bass
import concourse.mybir as mybir
import concourse.tile as tile
from concourse.bass_utils import run_bass_kernel_spmd

# Problem shapes (hardcoded per spec)
B = 16
C = 512
M = 1024  # 32*32 spatial
CC = 21
M2 = 16384  # 128*128 spatial
N_CORES = 8
BPC = B // N_CORES  # batches per core = 2

FP = mybir.dt.float32
FPR = mybir.dt.float32r
AX = mybir.AxisListType.X
ALU = mybir.AluOpType
ACTF = mybir.ActivationFunctionType

# IC m-axis split: 128 blocks of 128 -> u-chunks of 43/43/42 blocks per batch.
IC_BLOCKS = (43, 43, 42)
IC_LONG = 43 * 128  # 5504
IC_SHORT = 42 * 128  # 5376
# partition offsets of the 6 (batch, u) groups in the [126, *] staging tile
# order: (b0,u0) (b0,u1) (b1,u0) (b1,u1) (b0,u2) (b1,u2)
IC_GROUPS = [(0, 0, 0), (0, 1, 21), (1, 0, 42), (1, 1, 63), (0, 2, 84), (1, 2, 105)]


def _split_sync_waits(nc, cap=1):
    """walrus in this container accepts at most `cap` sync waits per
    instruction; hoist excess waits onto same-engine NOPs just before."""
    n = 0
    for f in nc.m.functions:
        for bb in f.blocks:
            newlist = []
            for ins in bb.instructions:
                si = ins.sync_info
                if si is not None and si.on_wait and len(si.on_wait) > cap:
                    waits = list(si.on_wait)
                    hoist, keep = waits[:-cap], waits[-cap:]
                    for w in hoist:
                        n += 1
                        newlist.append(
                            mybir.InstNoOp(
                                name=f"waitsplit-{n}",
                                engine=ins.engine,
                                ins=[],
                                outs=[],
                                sync_info=mybir.SyncInfo(on_wait=[w], on_update=[]),
                            )
                        )
                    si.on_wait = keep
                newlist.append(ins)
            bb.instructions = newlist


def _build():
    nc = bass.Bass(trn_type="TRN2")
    tf = nc.dram_tensor("TF", [BPC, C, M], FP, kind="ExternalInput")
    sf = nc.dram_tensor("SF", [BPC, C, M], FP, kind="ExternalInput")
    to = nc.dram_tensor("TO", [BPC, CC, M2], FP, kind="ExternalInput")
    so = nc.dram_tensor("SO", [BPC, CC, M2], FP, kind="ExternalInput")
    eye = nc.dram_tensor("EYE", [128, 128], FP, kind="ExternalInput")
    osa = nc.dram_tensor("OSA", [128, 1], FP, kind="ExternalOutput")
    oic = nc.dram_tensor("OIC", [CC, 2], FP, kind="ExternalOutput")

    with tile.TileContext(nc) as tc:
        with (
            tc.tile_pool(name="const", bufs=1) as pconst,
            tc.tile_pool(name="icl", bufs=1) as picl,
            tc.tile_pool(name="trs", bufs=3) as ptrs,
            tc.tile_pool(name="vt", bufs=1) as pv,
            tc.tile_pool(name="scr", bufs=3) as pscr,
            tc.tile_pool(name="stat", bufs=1) as pstat,
        ):
            eye_t = pconst.tile([128, 128], FP, name="eye_t")
            nc.sync.dma_start(eye_t[:], eye[:])

            # ---------------- SA input prep (no PSUM needed) ----------------
            # F staged in f32, V = F * sqrt(Fc) written as float32r (the
            # rounding producer the FP32r matmul verifier requires).
            vmats = {}
            for tname, src in (("t", tf), ("s", sf)):
                for bi in range(BPC):
                    fc = pstat.tile([128, 4], FP, name=f"fc_{tname}_{bi}")
                    sfc = pstat.tile([128, 4], FP, name=f"sfc_{tname}_{bi}")
                    for k in range(C // 128):
                        f = pscr.tile([128, M], FP, name=f"fstg_{k}", tag="fstg")
                        nc.sync.dma_start(
                            f[:], src[bi, 128 * k : 128 * (k + 1), :]
                        )
                        nc.vector.tensor_reduce(
                            out=fc[:, k : k + 1],
                            in_=f[:],
                            axis=AX,
                            op=ALU.add,
                            apply_absolute_value=True,
                        )
                        nc.scalar.activation(
                            sfc[:, k : k + 1], fc[:, k : k + 1], ACTF.Sqrt
                        )
                        v = pv.tile([128, M], FPR, name=f"v_{tname}_{bi}_{k}")
                        nc.vector.tensor_scalar_mul(
                            v[:], f[:], sfc[:, k : k + 1]
                        )
                        vmats[(tname, bi, k)] = v

            # ---------------- IC phase (own PSUM pools) ----------------
            lmats = {}
            for tname, src in (("t", to), ("s", so)):
                lt = picl.tile([126, IC_LONG], FP, name=f"icl_{tname}")
                for b, u, off in IC_GROUPS:
                    mstart = u * IC_LONG
                    mlen = IC_BLOCKS[u] * 128
                    nc.sync.dma_start(
                        lt[off : off + CC, 0:mlen],
                        src[b, :, mstart : mstart + mlen],
                    )
                lmats[tname] = lt

            ic_stats = {}  # (stat) -> [21, 2] tiles
            with (
                tc.tile_pool(name="ictr", bufs=2, space="PSUM") as ptr,
                tc.tile_pool(name="icg", bufs=1, space="PSUM") as pg,
            ):
                # psG: 4 accumulation chains [21, 21] at col offsets:
                # (t,b0)=0, (t,b1)=21, (s,b0)=42, (s,b1)=63
                psg = pg.tile([CC, 84], FP, name="psg")
                chain_col = {("t", 0): 0, ("t", 1): 21, ("s", 0): 42, ("s", 1): 63}
                chain_first = {k: True for k in chain_col}
                # count matmuls per chain to set stop flags: 128 blocks each
                chain_total = {k: 128 for k in chain_col}
                chain_done = {k: 0 for k in chain_col}

                for tname in ("t", "s"):
                    lt = lmats[tname]
                    nblocks = 43  # j index; j=42 only covers long groups
                    j = 0
                    while j < nblocks:
                        # pack up to 4 transposed blocks into one PSUM tile
                        pack = min(4, nblocks - j)
                        trp = ptr.tile([128, 504], FP, name="trp")
                        trs = ptrs.tile([128, 504], FP, name="trs")
                        for q in range(pack):
                            jj = j + q
                            npart = 126 if jj < 42 else 84
                            nc.tensor.transpose(
                                trp[:, 126 * q : 126 * q + npart],
                                lt[0:npart, 128 * jj : 128 * (jj + 1)],
                                eye_t[0:npart, 0:npart],
                            )
                        nc.scalar.copy(trs[:], trp[:])
                        for q in range(pack):
                            jj = j + q
                            groups = IC_GROUPS if jj < 42 else IC_GROUPS[:4]
                            for b, u, off in groups:
                                key = (tname, b)
                                col = chain_col[key]
                                chain_done[key] += 1
                                nc.tensor.matmul(
                                    psg[:, col : col + CC],
                                    lhsT=trs[:, 126 * q + off : 126 * q + off + CC],
                                    rhs=trs[:, 126 * q + off : 126 * q + off + CC],
                                    start=chain_first[key],
                                    stop=(chain_done[key] == chain_total[key]),
                                )
                                chain_first[key] = False
                        j += pack

                # stats: copy psG to SBUF, then row stats per batch
                gsb = pstat.tile([CC, 84], FP, name="gsb")
                nc.scalar.copy(gsb[:], psg[:])
                st_ic = pstat.tile([CC, 2], FP, name="st_ic")
                ns2_ic = pstat.tile([CC, 2], FP, name="ns2_ic")
                nt2_ic = pstat.tile([CC, 2], FP, name="nt2_ic")
                scr_ic = pstat.tile([CC, CC], FP, name="scr_ic")
                scr_ic2 = pstat.tile([CC, CC], FP, name="scr_ic2")
                for bi in range(BPC):
                    ct = chain_col[("t", bi)]
                    cs = chain_col[("s", bi)]
                    nc.vector.tensor_tensor(
                        scr_ic[:], gsb[:, cs : cs + CC], gsb[:, ct : ct + CC],
                        op=ALU.mult,
                    )
                    nc.vector.tensor_reduce(
                        out=st_ic[:, bi : bi + 1],
                        in_=scr_ic[:],
                        axis=AX,
                        op=ALU.add,
                    )
                    nc.scalar.activation(
                        scr_ic2[:],
                        gsb[:, cs : cs + CC],
                        ACTF.Square,
                        accum_out=ns2_ic[:, bi : bi + 1],
                    )
                    nc.scalar.activation(
                        scr_ic2[:],
                        gsb[:, ct : ct + CC],
                        ACTF.Square,
                        accum_out=nt2_ic[:, bi : bi + 1],
                    )
                ic_stats = (st_ic, ns2_ic, nt2_ic)

            # final IC math: rho = st / sqrt(ns2*nt2) -> OIC
            st_ic, ns2_ic, nt2_ic = ic_stats
            rn_ic = pstat.tile([CC, 2], FP, name="rn_ic")
            nc.vector.tensor_tensor(rn_ic[:], ns2_ic[:], nt2_ic[:], op=ALU.mult)
            nc.scalar.activation(rn_ic[:], rn_ic[:], ACTF.Sqrt)
            nc.vector.reciprocal(rn_ic[:], rn_ic[:])
            rho_ic = pstat.tile([CC, 2], FP, name="rho_ic")
            nc.vector.tensor_tensor(rho_ic[:], st_ic[:], rn_ic[:], op=ALU.mult)
            nc.sync.dma_start(oic[:], rho_ic[:])

            # ---------------- SA phase ----------------
            ns2b = pstat.tile([128, 16], FP, name="ns2b")
            nt2b = pstat.tile([128, 16], FP, name="nt2b")
            stb = pstat.tile([128, 16], FP, name="stb")
            with tc.tile_pool(name="saps", bufs=2, space="PSUM") as pps:
                for bi in range(BPC):
                    for mt in range(8):
                        idx = bi * 8 + mt
                        ps = pps.tile([128, 2048], FP, name="ps")
                        psa = ps[:, 0:M]  # A_T
                        psb = ps[:, M : 2 * M]  # A_S
                        for dst, tname in ((psa, "t"), (psb, "s")):
                            for nh in range(2):
                                for k in range(4):
                                    v = vmats[(tname, bi, k)]
                                    nc.tensor.matmul(
                                        dst[:, 512 * nh : 512 * (nh + 1)],
                                        lhsT=v[:, 128 * mt : 128 * (mt + 1)],
                                        rhs=v[:, 512 * nh : 512 * (nh + 1)],
                                        start=(k == 0),
                                        stop=(k == 3),
                                    )
                        at_sb = pscr.tile([128, M], FP, name="at_sb", tag="atsb")
                        scr1 = pscr.tile([128, M], FP, name="scr1", tag="scr")
                        scr2 = pscr.tile([128, M], FP, name="scr2", tag="scr")
                        scr3 = pscr.tile([128, M], FP, name="scr3", tag="scr")
                        # A_T to SBUF so later ops touch PSUM at most once each
                        nc.scalar.copy(at_sb[:], psa[:])
                        nc.scalar.activation(
                            scr1[:], psb[:], ACTF.Square,
                            accum_out=ns2b[:, idx : idx + 1],
                        )
                        nc.scalar.activation(
                            scr2[:], at_sb[:], ACTF.Square,
                            accum_out=nt2b[:, idx : idx + 1],
                        )
                        nc.vector.tensor_tensor(
                            scr3[:], psb[:], at_sb[:], op=ALU.mult
                        )
                        nc.vector.tensor_reduce(
                            out=stb[:, idx : idx + 1],
                            in_=scr3[:],
                            axis=AX,
                            op=ALU.add,
                        )

            # final SA math: rho rows -> per-partition sums -> OSA
            rn = pstat.tile([128, 16], FP, name="rn")
            nc.vector.tensor_tensor(rn[:], ns2b[:], nt2b[:], op=ALU.mult)
            nc.scalar.activation(rn[:], rn[:], ACTF.Sqrt)
            nc.vector.reciprocal(rn[:], rn[:])
            prho = pstat.tile([128, 16], FP, name="prho")
            nc.vector.tensor_tensor(prho[:], stb[:], rn[:], op=ALU.mult)
            osa_t = pstat.tile([128, 1], FP, name="osa_t")
            nc.vector.tensor_reduce(
                out=osa_t[:], in_=prho[:], axis=AX, op=ALU.add
            )
            nc.sync.dma_start(osa[:], osa_t[:])

    _split_sync_waits(nc)
    return nc


_NC = None


def _get_nc():
    global _NC
    if _NC is None:
        _NC = _build()
    return _NC


_EYE = np.eye(128, dtype=np.float32)


def _make_in_maps(TF, SF, t_out, s_out):
    TFr = np.ascontiguousarray(TF.reshape(B, C, M), dtype=np.float32)
    SFr = np.ascontiguousarray(SF.reshape(B, C, M), dtype=np.float32)
    TOr = np.ascontiguousarray(t_out.reshape(B, CC, M2), dtype=np.float32)
    SOr = np.ascontiguousarray(s_out.reshape(B, CC, M2), dtype=np.float32)
    in_maps = []
    for i in range(N_CORES):
        sl = slice(BPC * i, BPC * (i + 1))
        in_maps.append(
            {
                "TF": np.ascontiguousarray(TFr[sl]),
                "SF": np.ascontiguousarray(SFr[sl]),
                "TO": np.ascontiguousarray(TOr[sl]),
                "SO": np.ascontiguousarray(SOr[sl]),
                "EYE": _EYE,
            }
        )
    return in_maps


def _assemble(results, s_out):
    sa_rho = 0.0
    ic_rho = 0.0
    for r in results:
        sa_rho += float(r["OSA"].astype(np.float64).sum())
        ic_rho += float(r["OIC"].astype(np.float64).sum())
    sa_loss = (2.0 * B * M - 2.0 * sa_rho) / (B * M * M)
    ic_loss = (2.0 * B * CC - 2.0 * ic_rho) / (CC * B)
    return (s_out, np.float32(ic_loss), np.float32(sa_loss))


def kernel(TF, SF, t_out, s_out, _trace=False):
    nc = _get_nc()
    in_maps = _make_in_maps(TF, SF, t_out, s_out)
    res = run_bass_kernel_spmd(nc, in_maps, core_ids=list(range(N_CORES)), trace=_trace)
    out = _assemble(res.results, s_out)
    if _trace:
        return out, res
    return out


# revision 9
# speedup vs baseline: 1.1099x; 1.1099x over previous
"""Trainium2 Bass kernel for nn_Distiller (attention-transfer distillation loss).

Computes on 8 NeuronCores (data-parallel over batch, 2 batches/core):
  SA part: per batch, weighted spatial grams A = V^T V with V = F * sqrt(Fc)
           (Fc = sum |F| over space), for teacher/student features; then
           rho_m = <A_S[m], A_T[m]> / (|A_S[m]| |A_T[m]|) per row.
           Uses the identity sum_n (Ahat_S - Ahat_T)^2 = 2 - 2*rho per row.
  IC part: per batch, channel grams G = L L^T of the [21, 16384] logit maps,
           same rho identity on the 21 rows of G.
Device emits per-row rho partial sums; host assembles the two scalar losses.
s_out passes through on host.
"""

import sys

if "/opt/trn_rl_repo" not in sys.path:
    sys.path.insert(0, "/opt/trn_rl_repo")

import numpy as np

import concourse.bass as bass
import concourse.mybir as mybir
import concourse.tile as tile
from concourse.bass_utils import run_bass_kernel_spmd

# Problem shapes (hardcoded per spec)
B = 16
C = 512
M = 1024  # 32*32 spatial
CC = 21
M2 = 16384  # 128*128 spatial
N_CORES = 8
BPC = B // N_CORES  # batches per core = 2

FP = mybir.dt.float32
FPR = mybir.dt.float32r
AX = mybir.AxisListType.X
ALU = mybir.AluOpType
ACTF = mybir.ActivationFunctionType

# IC m-axis split: 128 blocks of 128 -> u-chunks of 43/43/42 blocks per batch.
IC_BLOCKS = (43, 43, 42)
IC_LONG = 43 * 128  # 5504
# partition offsets of the 6 (batch, u) groups in the [126, *] staging tile
# order: (b0,u0) (b0,u1) (b1,u0) (b1,u1) (b0,u2) (b1,u2)
IC_GROUPS = [(0, 0, 0), (0, 1, 21), (1, 0, 42), (1, 1, 63), (0, 2, 84), (1, 2, 105)]


def _split_sync_waits(nc, cap=1):
    """walrus in this container accepts at most `cap` sync waits per
    instruction; hoist excess waits onto same-engine NOPs just before."""
    n = 0
    for f in nc.m.functions:
        for bb in f.blocks:
            newlist = []
            for ins in bb.instructions:
                si = ins.sync_info
                if si is not None and si.on_wait and len(si.on_wait) > cap:
                    waits = list(si.on_wait)
                    hoist, keep = waits[:-cap], waits[-cap:]
                    for w in hoist:
                        n += 1
                        newlist.append(
                            mybir.InstNoOp(
                                name=f"waitsplit-{n}",
                                engine=ins.engine,
                                ins=[],
                                outs=[],
                                sync_info=mybir.SyncInfo(on_wait=[w], on_update=[]),
                            )
                        )
                    si.on_wait = keep
                newlist.append(ins)
            bb.instructions = newlist


def _build():
    nc = bass.Bass(trn_type="TRN2")
    tf = nc.dram_tensor("TF", [BPC, C, M], FP, kind="ExternalInput")
    sf = nc.dram_tensor("SF", [BPC, C, M], FP, kind="ExternalInput")
    to = nc.dram_tensor("TO", [BPC, CC, M2], FP, kind="ExternalInput")
    so = nc.dram_tensor("SO", [BPC, CC, M2], FP, kind="ExternalInput")
    eye = nc.dram_tensor("EYE", [128, 128], FP, kind="ExternalInput")
    osa = nc.dram_tensor("OSA", [128, 1], FP, kind="ExternalOutput")
    oic = nc.dram_tensor("OIC", [CC, 2], FP, kind="ExternalOutput")

    with tile.TileContext(nc) as tc:
        with (
            tc.tile_pool(name="const", bufs=1) as pconst,
            tc.tile_pool(name="icl", bufs=1) as picl,
            tc.tile_pool(name="trs", bufs=3) as ptrs,
            tc.tile_pool(name="vt", bufs=1) as pv,
            tc.tile_pool(name="fstg", bufs=8) as pf,
            tc.tile_pool(name="scr", bufs=3) as pscr,
            tc.tile_pool(name="stat", bufs=1) as pstat,
        ):
            eye_t = pconst.tile([128, 128], FP, name="eye_t")
            nc.sync.dma_start(eye_t[:], eye[:])

            # ---- IC loads first: PE's first work (transposes) needs these,
            # and SP dispatches DMAs in program order.
            lmats = {}
            for tname, src in (("t", to), ("s", so)):
                lt = picl.tile([126, IC_LONG], FP, name=f"icl_{tname}")
                for b, u, off in IC_GROUPS:
                    mstart = u * IC_LONG
                    mlen = IC_BLOCKS[u] * 128
                    nc.sync.dma_start(
                        lt[off : off + CC, 0:mlen],
                        src[b, :, mstart : mstart + mlen],
                    )
                lmats[tname] = lt

            # ---------------- SA input prep (no PSUM needed) ----------------
            # F staged in f32, V = F * sqrt(Fc) written as float32r (the
            # rounding producer the FP32r matmul verifier requires).
            vmats = {}
            for tname, src in (("t", tf), ("s", sf)):
                for bi in range(BPC):
                    fc = pstat.tile([128, 4], FP, name=f"fc_{tname}_{bi}")
                    sfc = pstat.tile([128, 4], FP, name=f"sfc_{tname}_{bi}")
                    for k in range(C // 128):
                        f = pf.tile([128, M], FP, name=f"fstg_{k}", tag="fstg")
                        nc.sync.dma_start(
                            f[:], src[bi, 128 * k : 128 * (k + 1), :]
                        )
                        nc.vector.tensor_reduce(
                            out=fc[:, k : k + 1],
                            in_=f[:],
                            axis=AX,
                            op=ALU.add,
                            apply_absolute_value=True,
                        )
                        nc.scalar.activation(
                            sfc[:, k : k + 1], fc[:, k : k + 1], ACTF.Sqrt
                        )
                        v = pv.tile([128, M], FPR, name=f"v_{tname}_{bi}_{k}")
                        nc.vector.tensor_scalar_mul(
                            v[:], f[:], sfc[:, k : k + 1]
                        )
                        vmats[(tname, bi, k)] = v

            # ---------------- IC phase (own PSUM pools) ----------------
            ic_stats = {}
            with (
                tc.tile_pool(name="ictr", bufs=2, space="PSUM") as ptr,
                tc.tile_pool(name="icg", bufs=1, space="PSUM") as pg,
            ):
                # psG: 4 accumulation chains [21, 21] at col offsets:
                # (t,b0)=0, (t,b1)=21, (s,b0)=42, (s,b1)=63
                # f32r matmul needs an even moving free dim: rhs padded to 22
                # cols (garbage 22nd column), chains strided 22 apart.
                psg = pg.tile([CC, 88], FP, name="psg")
                chain_col = {("t", 0): 0, ("t", 1): 22, ("s", 0): 44, ("s", 1): 66}
                chain_first = {k: True for k in chain_col}
                chain_total = {k: 128 for k in chain_col}
                chain_done = {k: 0 for k in chain_col}

                for tname in ("t", "s"):
                    lt = lmats[tname]
                    nblocks = 43  # j index; j=42 only covers long groups
                    j = 0
                    while j < nblocks:
                        # pack up to 4 transposed blocks into one PSUM tile
                        pack = min(4, nblocks - j)
                        trp = ptr.tile([128, 504], FP, name="trp")
                        # f32r copy = rounding producer for the gram matmuls
                        trs = ptrs.tile([128, 512], FPR, name="trs")
                        for q in range(pack):
                            jj = j + q
                            npart = 126 if jj < 42 else 84
                            nc.tensor.transpose(
                                trp[:, 126 * q : 126 * q + npart],
                                lt[0:npart, 128 * jj : 128 * (jj + 1)],
                                eye_t[0:npart, 0:npart],
                            )
                        nc.scalar.copy(trs[:, 0:504], trp[:])
                        for q in range(pack):
                            jj = j + q
                            groups = IC_GROUPS if jj < 42 else IC_GROUPS[:4]
                            for b, u, off in groups:
                                key = (tname, b)
                                col = chain_col[key]
                                chain_done[key] += 1
                                nc.tensor.matmul(
                                    psg[:, col : col + 22],
                                    lhsT=trs[:, 126 * q + off : 126 * q + off + CC],
                                    rhs=trs[:, 126 * q + off : 126 * q + off + 22],
                                    start=chain_first[key],
                                    stop=(chain_done[key] == chain_total[key]),
                                )
                                chain_first[key] = False
                        j += pack

                # stats: copy psG to SBUF, then row stats per batch
                gsb = pstat.tile([CC, 88], FP, name="gsb")
                nc.scalar.copy(gsb[:], psg[:])
                st_ic = pstat.tile([CC, 2], FP, name="st_ic")
                ns2_ic = pstat.tile([CC, 2], FP, name="ns2_ic")
                nt2_ic = pstat.tile([CC, 2], FP, name="nt2_ic")
                scr_ic = pstat.tile([CC, CC], FP, name="scr_ic")
                scr_ic2 = pstat.tile([CC, CC], FP, name="scr_ic2")
                for bi in range(BPC):
                    ct = chain_col[("t", bi)]
                    cs = chain_col[("s", bi)]
                    nc.vector.scalar_tensor_tensor(
                        out=scr_ic[:],
                        in0=gsb[:, cs : cs + CC],
                        scalar=1.0,
                        in1=gsb[:, ct : ct + CC],
                        op0=ALU.mult,
                        op1=ALU.mult,
                        accum_out=st_ic[:, bi : bi + 1],
                    )
                    nc.scalar.activation(
                        scr_ic2[:],
                        gsb[:, cs : cs + CC],
                        ACTF.Square,
                        accum_out=ns2_ic[:, bi : bi + 1],
                    )
                    nc.scalar.activation(
                        scr_ic2[:],
                        gsb[:, ct : ct + CC],
                        ACTF.Square,
                        accum_out=nt2_ic[:, bi : bi + 1],
                    )
                ic_stats = (st_ic, ns2_ic, nt2_ic)

            # final IC math: rho = st / sqrt(ns2*nt2) -> OIC
            st_ic, ns2_ic, nt2_ic = ic_stats
            rn_ic = pstat.tile([CC, 2], FP, name="rn_ic")
            nc.vector.tensor_tensor(rn_ic[:], ns2_ic[:], nt2_ic[:], op=ALU.mult)
            nc.scalar.activation(rn_ic[:], rn_ic[:], ACTF.Sqrt)
            nc.vector.reciprocal(rn_ic[:], rn_ic[:])
            rho_ic = pstat.tile([CC, 2], FP, name="rho_ic")
            nc.vector.tensor_tensor(rho_ic[:], st_ic[:], rn_ic[:], op=ALU.mult)
            nc.sync.dma_start(oic[:], rho_ic[:])

            # ---------------- SA phase ----------------
            ns2b = pstat.tile([128, 16], FP, name="ns2b")
            nt2b = pstat.tile([128, 16], FP, name="nt2b")
            stb = pstat.tile([128, 16], FP, name="stb")
            with tc.tile_pool(name="saps", bufs=2, space="PSUM") as pps:
                for bi in range(BPC):
                    for mt in range(8):
                        idx = bi * 8 + mt
                        ps = pps.tile([128, 2048], FP, name="ps")
                        psa = ps[:, 0:M]  # A_T
                        psb = ps[:, M : 2 * M]  # A_S
                        for dst, tname in ((psa, "t"), (psb, "s")):
                            for nh in range(2):
                                for k in range(4):
                                    v = vmats[(tname, bi, k)]
                                    nc.tensor.matmul(
                                        dst[:, 512 * nh : 512 * (nh + 1)],
                                        lhsT=v[:, 128 * mt : 128 * (mt + 1)],
                                        rhs=v[:, 512 * nh : 512 * (nh + 1)],
                                        start=(k == 0),
                                        stop=(k == 3),
                                    )
                        at_sb = pscr.tile([128, M], FP, name="at_sb", tag="atsb")
                        scr1 = pscr.tile([128, M], FP, name="scr1", tag="scr")
                        scr2 = pscr.tile([128, M], FP, name="scr2", tag="scr")
                        scr3 = pscr.tile([128, M], FP, name="scr3", tag="scr")
                        # A_T to SBUF so each op below touches PSUM at most once
                        nc.vector.tensor_copy(at_sb[:], psa[:])
                        nc.scalar.activation(
                            scr1[:], psb[:], ACTF.Square,
                            accum_out=ns2b[:, idx : idx + 1],
                        )
                        nc.scalar.activation(
                            scr2[:], at_sb[:], ACTF.Square,
                            accum_out=nt2b[:, idx : idx + 1],
                        )
                        nc.vector.scalar_tensor_tensor(
                            out=scr3[:],
                            in0=psb[:],
                            scalar=1.0,
                            in1=at_sb[:],
                            op0=ALU.mult,
                            op1=ALU.mult,
                            accum_out=stb[:, idx : idx + 1],
                        )

            # final SA math: rho rows -> per-partition sums -> OSA
            rn = pstat.tile([128, 16], FP, name="rn")
            nc.vector.tensor_tensor(rn[:], ns2b[:], nt2b[:], op=ALU.mult)
            nc.scalar.activation(rn[:], rn[:], ACTF.Sqrt)
            nc.vector.reciprocal(rn[:], rn[:])
            prho = pstat.tile([128, 16], FP, name="prho")
            nc.vector.tensor_tensor(prho[:], stb[:], rn[:], op=ALU.mult)
            osa_t = pstat.tile([128, 1], FP, name="osa_t")
            nc.vector.tensor_reduce(
                out=osa_t[:], in_=prho[:], axis=AX, op=ALU.add
            )
            nc.sync.dma_start(osa[:], osa_t[:])

    _split_sync_waits(nc)
    return nc


_NC = None


def _get_nc():
    global _NC
    if _NC is None:
        _NC = _build()
    return _NC


_EYE = np.eye(128, dtype=np.float32)


def _make_in_maps(TF, SF, t_out, s_out):
    TFr = np.ascontiguousarray(TF.reshape(B, C, M), dtype=np.float32)
    SFr = np.ascontiguousarray(SF.reshape(B, C, M), dtype=np.float32)
    TOr = np.ascontiguousarray(t_out.reshape(B, CC, M2), dtype=np.float32)
    SOr = np.ascontiguousarray(s_out.reshape(B, CC, M2), dtype=np.float32)
    in_maps = []
    for i in range(N_CORES):
        sl = slice(BPC * i, BPC * (i + 1))
        in_maps.append(
            {
                "TF": np.ascontiguousarray(TFr[sl]),
                "SF": np.ascontiguousarray(SFr[sl]),
                "TO": np.ascontiguousarray(TOr[sl]),
                "SO": np.ascontiguousarray(SOr[sl]),
                "EYE": _EYE,
            }
        )
    return in_maps


def _assemble(results, s_out):
    sa_rho = 0.0
    ic_rho = 0.0
    for r in results:
        sa_rho += float(r["OSA"].astype(np.float64).sum())
        ic_rho += float(r["OIC"].astype(np.float64).sum())
    sa_loss = (2.0 * B * M - 2.0 * sa_rho) / (B * M * M)
    ic_loss = (2.0 * B * CC - 2.0 * ic_rho) / (CC * B)
    return (s_out, np.float32(ic_loss), np.float32(sa_loss))


def kernel(TF, SF, t_out, s_out, _trace=False):
    nc = _get_nc()
    in_maps = _make_in_maps(TF, SF, t_out, s_out)
    res = run_bass_kernel_spmd(nc, in_maps, core_ids=list(range(N_CORES)), trace=_trace)
    out = _assemble(res.results, s_out)
    if _trace:
        return out, res
    return out


# revision 15
# speedup vs baseline: 1.1588x; 1.0441x over previous
"""Trainium2 Bass kernel for nn_Distiller (attention-transfer distillation loss).

Computes on 8 NeuronCores (data-parallel over batch, 2 batches/core):
  SA part: per batch, weighted spatial grams A = V^T V with V = F * sqrt(Fc)
           (Fc = sum |F| over space), for teacher/student features; then
           rho_m = <A_S[m], A_T[m]> / (|A_S[m]| |A_T[m]|) per row.
           Uses the identity sum_n (Ahat_S - Ahat_T)^2 = 2 - 2*rho per row.
  IC part: per batch, channel grams G = L L^T of the [21, 16384] logit maps,
           same rho identity on the 21 rows of G.
Device emits per-row rho partial sums; host assembles the two scalar losses.
s_out passes through on host.
"""

import sys

if "/opt/trn_rl_repo" not in sys.path:
    sys.path.insert(0, "/opt/trn_rl_repo")

import numpy as np
import ml_dtypes

import concourse.bass as bass
import concourse.mybir as mybir
import concourse.tile as tile
from concourse.bass_utils import run_bass_kernel_spmd

# Problem shapes (hardcoded per spec)
B = 16
C = 512
M = 1024  # 32*32 spatial
CC = 21
M2 = 16384  # 128*128 spatial
N_CORES = 8
BPC = B // N_CORES  # batches per core = 2

FP = mybir.dt.float32
FPR = mybir.dt.float32r
BF = mybir.dt.bfloat16
AX = mybir.AxisListType.X
ALU = mybir.AluOpType
ACTF = mybir.ActivationFunctionType

# IC m-axis split: 128 blocks of 128 -> u-chunks of 43/43/42 blocks per batch.
IC_BLOCKS = (43, 43, 42)
IC_LONG = 43 * 128  # 5504
# partition offsets of the 6 (batch, u) groups in the [126, *] staging tile
# order: (b0,u0) (b0,u1) (b1,u0) (b1,u1) (b0,u2) (b1,u2)
IC_GROUPS = [(0, 0, 0), (0, 1, 21), (1, 0, 42), (1, 1, 63), (0, 2, 84), (1, 2, 105)]


def _split_sync_waits(nc, cap=1):
    """walrus in this container accepts at most `cap` sync waits per
    instruction; hoist excess waits onto same-engine NOPs just before."""
    n = 0
    for f in nc.m.functions:
        for bb in f.blocks:
            newlist = []
            for ins in bb.instructions:
                si = ins.sync_info
                if si is not None and si.on_wait and len(si.on_wait) > cap:
                    waits = list(si.on_wait)
                    hoist, keep = waits[:-cap], waits[-cap:]
                    for w in hoist:
                        n += 1
                        newlist.append(
                            mybir.InstNoOp(
                                name=f"waitsplit-{n}",
                                engine=ins.engine,
                                ins=[],
                                outs=[],
                                sync_info=mybir.SyncInfo(on_wait=[w], on_update=[]),
                            )
                        )
                    si.on_wait = keep
                newlist.append(ins)
            bb.instructions = newlist


def _build():
    nc = bass.Bass(trn_type="TRN2")
    tf = nc.dram_tensor("TF", [BPC, C, M], FP, kind="ExternalInput")
    sf = nc.dram_tensor("SF", [BPC, C, M], FP, kind="ExternalInput")
    # IC inputs: bf16 hi/lo split, both batches packed into 128 rows:
    # rows 0:21 b0-hi, 32:53 b0-lo, 64:85 b1-hi, 96:117 b1-lo (32-aligned so
    # PE-transpose fixups of the lo^T-hi block are tile_position-legal).
    to = nc.dram_tensor("TOHL", [128, M2], BF, kind="ExternalInput")
    so = nc.dram_tensor("SOHL", [128, M2], BF, kind="ExternalInput")
    eye = nc.dram_tensor("EYE", [128, 128], FP, kind="ExternalInput")
    osa = nc.dram_tensor("OSA", [128, 1], FP, kind="ExternalOutput")
    oic = nc.dram_tensor("OIC", [CC, 2], FP, kind="ExternalOutput")

    with tile.TileContext(nc) as tc:
        with (
            tc.tile_pool(name="const", bufs=1) as pconst,
            tc.tile_pool(name="icl", bufs=1) as picl,
            tc.tile_pool(name="trs", bufs=3) as ptrs,
            tc.tile_pool(name="vt", bufs=1) as pv,
            tc.tile_pool(name="fstg", bufs=8) as pf,
            tc.tile_pool(name="scr", bufs=3) as pscr,
            tc.tile_pool(name="stat", bufs=1) as pstat,
        ):
            eye_t = pconst.tile([128, 128], FP, name="eye_t")
            nc.sync.dma_start(eye_t[:], eye[:])

            # ---- IC loads first (PE's first work needs them; SP dispatches
            # DMAs in program order). Hardware DMA-transpose: [128, 4096]
            # chunks land as [128p, 32blk, 128ch] with m = blk*128 + p.
            lmats = {}
            for tname, hl in (("t", to), ("s", so)):
                tt = picl.tile([128, 128, 128], BF, name=f"tt_{tname}")
                for c in range(4):
                    nc.sync.dma_start_transpose(
                        tt[:, 32 * c : 32 * (c + 1), :],
                        hl[:, 4096 * c : 4096 * (c + 1)],
                    )
                lmats[tname] = tt

            # ---------------- SA input prep (no PSUM needed) ----------------
            # F staged in f32, V = F * sqrt(Fc) written as float32r (the
            # rounding producer the FP32r matmul verifier requires).
            vmats = {}
            for tname, src in (("t", tf), ("s", sf)):
                for bi in range(BPC):
                    fc = pstat.tile([128, 4], FP, name=f"fc_{tname}_{bi}")
                    sfc = pstat.tile([128, 4], FP, name=f"sfc_{tname}_{bi}")
                    for k in range(C // 128):
                        f = pf.tile([128, M], FP, name=f"fstg_{k}", tag="fstg")
                        nc.sync.dma_start(
                            f[:], src[bi, 128 * k : 128 * (k + 1), :]
                        )
                        nc.vector.tensor_reduce(
                            out=fc[:, k : k + 1],
                            in_=f[:],
                            axis=AX,
                            op=ALU.add,
                            apply_absolute_value=True,
                        )
                        nc.scalar.activation(
                            sfc[:, k : k + 1], fc[:, k : k + 1], ACTF.Sqrt
                        )
                        v = pv.tile([128, M], FPR, name=f"v_{tname}_{bi}_{k}")
                        nc.vector.tensor_scalar_mul(
                            v[:], f[:], sfc[:, k : k + 1]
                        )
                        vmats[(tname, bi, k)] = v

            # ---------------- IC phase (own PSUM pools) ----------------
            # One matmul per (tensor, m-block): lhsT = rhs = TT[:, j, :] so the
            # [118, 118] product holds hi/lo cross-grams of both batches.
            # G_b = hh + hl + (hl)^T per batch (ll term ~2^-18, dropped... ll
            # included via nothing; hh/hl exact bf16 products, fp32 accum).
            with (
                tc.tile_pool(name="icg", bufs=1, space="PSUM") as pg,
                tc.tile_pool(name="icfix", bufs=1, space="PSUM") as pfx,
            ):
                psgs = {}
                for tname in ("t", "s"):
                    tt = lmats[tname]
                    psg = pg.tile([128, 118], FP, name=f"psg_{tname}")
                    for j in range(128):
                        nc.tensor.matmul(
                            psg[0:117, :],
                            lhsT=tt[:, j, 0:117],
                            rhs=tt[:, j, 0:118],
                            start=(j == 0),
                            stop=(j == 127),
                        )
                    psgs[tname] = psg

                # blocks per batch: b0 hh=(0:21,0:21) hl=(0:21,32:53)
                #                   b1 hh=(64:85,64:85) hl=(64:85,96:117)
                HH = {0: (0, 0), 1: (64, 64)}
                HL = {0: (0, 32), 1: (64, 96)}
                LH = {0: (32, 0), 1: (96, 64)}
                gsbs = {}
                hlts = {}
                for tname in ("t", "s"):
                    gsb = pstat.tile([128, 118], FP, name=f"gsb_{tname}")
                    nc.scalar.copy(gsb[:], psgs[tname][:])
                    gsbs[tname] = gsb
                    # fix-up: lh contribution = (hl)^T; transpose the hl
                    # block (rows at base 0/64 are tile_position-legal inputs).
                    # Each transpose gets its own PSUM tile (same-tile
                    # column-disjoint transpose outputs fault at runtime), and
                    # b1's block is realigned to partition 64 via an identity
                    # matmul.
                    hlt = pstat.tile([128, 21], FP, name=f"hlt_{tname}")
                    pstr0 = pfx.tile([128, 21], FP, name=f"pstr0_{tname}")
                    rp, cp = HL[0]
                    nc.tensor.transpose(
                        pstr0[0:21, 0:21],
                        gsb[rp : rp + 21, cp : cp + 21],
                        eye_t[rp : rp + 21, rp : rp + 21],
                    )
                    nc.scalar.copy(hlt[0:21, :], pstr0[0:21, :])
                    pstr1 = pfx.tile([128, 21], FP, name=f"pstr1_{tname}")
                    rp, cp = HL[1]
                    nc.tensor.transpose(
                        pstr1[0:21, 0:21],
                        gsb[rp : rp + 21, cp : cp + 21],
                        eye_t[rp : rp + 21, rp : rp + 21],
                    )
                    h1sb = pstat.tile([128, 21], FP, name=f"h1sb_{tname}")
                    nc.scalar.copy(h1sb[0:21, :], pstr1[0:21, :])
                    ps2 = pfx.tile([128, 21], FP, name=f"ps2_{tname}")
                    nc.tensor.matmul(
                        ps2[64:85, :],
                        lhsT=eye_t[0:21, 0:21],
                        rhs=h1sb[0:21, 0:21],
                        start=True,
                        stop=True,
                    )
                    nc.scalar.copy(hlt[64:85, :], ps2[64:85, :])
                    hlts[tname] = hlt

                # G_b = hh + hl + hlT, then row stats; all per-batch tiles sit
                # at partition base 64*bi so DVE lanes line up.
                st_ic = pstat.tile([128, 2], FP, name="st_ic")
                ns2_ic = pstat.tile([128, 2], FP, name="ns2_ic")
                nt2_ic = pstat.tile([128, 2], FP, name="nt2_ic")
                gsum = {}
                for tname in ("t", "s"):
                    g = pstat.tile([128, 21], FP, name=f"gsum_{tname}")
                    for bi in range(BPC):
                        rp, _ = HH[bi]
                        hh = gsbs[tname][rp : rp + 21, HH[bi][1] : HH[bi][1] + 21]
                        hl = gsbs[tname][rp : rp + 21, HL[bi][1] : HL[bi][1] + 21]
                        hlt = hlts[tname][64 * bi : 64 * bi + 21, 0:21]
                        gslice = g[64 * bi : 64 * bi + 21, :]
                        nc.vector.tensor_tensor(gslice, hh, hl, op=ALU.add)
                        nc.vector.tensor_tensor(gslice, gslice, hlt, op=ALU.add)
                    gsum[tname] = g
                scr_ic = pstat.tile([128, 21], FP, name="scr_ic")
                scr_ic2 = pstat.tile([128, 21], FP, name="scr_ic2")
                for bi in range(BPC):
                    o = 64 * bi
                    gs = gsum["s"][o : o + 21, :]
                    gt = gsum["t"][o : o + 21, :]
                    nc.vector.scalar_tensor_tensor(
                        out=scr_ic[o : o + 21, :],
                        in0=gs,
                        scalar=1.0,
                        in1=gt,
                        op0=ALU.mult,
                        op1=ALU.mult,
                        accum_out=st_ic[o : o + 21, bi : bi + 1],
                    )
                    nc.scalar.activation(
                        scr_ic2[o : o + 21, :], gs, ACTF.Square,
                        accum_out=ns2_ic[o : o + 21, bi : bi + 1],
                    )
                    nc.scalar.activation(
                        scr_ic2[o : o + 21, :], gt, ACTF.Square,
                        accum_out=nt2_ic[o : o + 21, bi : bi + 1],
                    )

            # final IC math: rho = st / sqrt(ns2*nt2) -> OIC
            rn_ic = pstat.tile([128, 2], FP, name="rn_ic")
            nc.vector.tensor_tensor(rn_ic[:], ns2_ic[:], nt2_ic[:], op=ALU.mult)
            nc.scalar.activation(rn_ic[:], rn_ic[:], ACTF.Sqrt)
            nc.vector.reciprocal(rn_ic[:], rn_ic[:])
            rho_ic = pstat.tile([128, 2], FP, name="rho_ic")
            nc.vector.tensor_tensor(rho_ic[:], st_ic[:], rn_ic[:], op=ALU.mult)
            nc.sync.dma_start(oic[:, 0:1], rho_ic[0:21, 0:1])
            nc.sync.dma_start(oic[:, 1:2], rho_ic[64:85, 1:2])

            # ---------------- SA phase ----------------
            ns2b = pstat.tile([128, 16], FP, name="ns2b")
            nt2b = pstat.tile([128, 16], FP, name="nt2b")
            stb = pstat.tile([128, 16], FP, name="stb")
            with tc.tile_pool(name="saps", bufs=2, space="PSUM") as pps:
                for bi in range(BPC):
                    for mt in range(8):
                        idx = bi * 8 + mt
                        ps = pps.tile([128, 2048], FP, name="ps")
                        psa = ps[:, 0:M]  # A_T
                        psb = ps[:, M : 2 * M]  # A_S
                        for dst, tname in ((psa, "t"), (psb, "s")):
                            for nh in range(2):
                                for k in range(4):
                                    v = vmats[(tname, bi, k)]
                                    nc.tensor.matmul(
                                        dst[:, 512 * nh : 512 * (nh + 1)],
                                        lhsT=v[:, 128 * mt : 128 * (mt + 1)],
                                        rhs=v[:, 512 * nh : 512 * (nh + 1)],
                                        start=(k == 0),
                                        stop=(k == 3),
                                    )
                        at_sb = pscr.tile([128, M], FP, name="at_sb", tag="atsb")
                        scr1 = pscr.tile([128, M], FP, name="scr1", tag="scr")
                        scr2 = pscr.tile([128, M], FP, name="scr2", tag="scr")
                        scr3 = pscr.tile([128, M], FP, name="scr3", tag="scr")
                        # A_T to SBUF so each op below touches PSUM at most once
                        nc.vector.tensor_copy(at_sb[:], psa[:])
                        nc.scalar.activation(
                            scr1[:], psb[:], ACTF.Square,
                            accum_out=ns2b[:, idx : idx + 1],
                        )
                        nc.scalar.activation(
                            scr2[:], at_sb[:], ACTF.Square,
                            accum_out=nt2b[:, idx : idx + 1],
                        )
                        nc.vector.scalar_tensor_tensor(
                            out=scr3[:],
                            in0=psb[:],
                            scalar=1.0,
                            in1=at_sb[:],
                            op0=ALU.mult,
                            op1=ALU.mult,
                            accum_out=stb[:, idx : idx + 1],
                        )

            # final SA math: rho rows -> per-partition sums -> OSA
            rn = pstat.tile([128, 16], FP, name="rn")
            nc.vector.tensor_tensor(rn[:], ns2b[:], nt2b[:], op=ALU.mult)
            nc.scalar.activation(rn[:], rn[:], ACTF.Sqrt)
            nc.vector.reciprocal(rn[:], rn[:])
            prho = pstat.tile([128, 16], FP, name="prho")
            nc.vector.tensor_tensor(prho[:], stb[:], rn[:], op=ALU.mult)
            osa_t = pstat.tile([128, 1], FP, name="osa_t")
            nc.vector.tensor_reduce(
                out=osa_t[:], in_=prho[:], axis=AX, op=ALU.add
            )
            nc.sync.dma_start(osa[:], osa_t[:])

    _split_sync_waits(nc)
    return nc


_NC = None


def _get_nc():
    global _NC
    if _NC is None:
        _NC = _build()
    return _NC


_EYE = np.eye(128, dtype=np.float32)


def _hl_pack(X):
    """[B, CC, M2] f32 -> per-core [128, M2] bf16 with rows
    0:21 b0-hi, 32:53 b0-lo, 64:85 b1-hi, 96:117 b1-lo."""
    bf = ml_dtypes.bfloat16
    hi = X.astype(bf)
    lo = (X - hi.astype(np.float32)).astype(bf)
    out = np.zeros((N_CORES, 128, M2), dtype=bf)
    out[:, 0:21] = hi[0::2]
    out[:, 32:53] = lo[0::2]
    out[:, 64:85] = hi[1::2]
    out[:, 96:117] = lo[1::2]
    return out


def _make_in_maps(TF, SF, t_out, s_out):
    TFr = np.ascontiguousarray(TF.reshape(B, C, M), dtype=np.float32)
    SFr = np.ascontiguousarray(SF.reshape(B, C, M), dtype=np.float32)
    TOhl = _hl_pack(np.asarray(t_out, dtype=np.float32).reshape(B, CC, M2))
    SOhl = _hl_pack(np.asarray(s_out, dtype=np.float32).reshape(B, CC, M2))
    in_maps = []
    for i in range(N_CORES):
        sl = slice(BPC * i, BPC * (i + 1))
        in_maps.append(
            {
                "TF": np.ascontiguousarray(TFr[sl]),
                "SF": np.ascontiguousarray(SFr[sl]),
                "TOHL": np.ascontiguousarray(TOhl[i]),
                "SOHL": np.ascontiguousarray(SOhl[i]),
                "EYE": _EYE,
            }
        )
    return in_maps


def _assemble(results, s_out):
    sa_rho = 0.0
    ic_rho = 0.0
    for r in results:
        sa_rho += float(r["OSA"].astype(np.float64).sum())
        ic_rho += float(r["OIC"].astype(np.float64).sum())
    sa_loss = (2.0 * B * M - 2.0 * sa_rho) / (B * M * M)
    ic_loss = (2.0 * B * CC - 2.0 * ic_rho) / (CC * B)
    return (s_out, np.float32(ic_loss), np.float32(sa_loss))


def kernel(TF, SF, t_out, s_out, _trace=False):
    nc = _get_nc()
    in_maps = _make_in_maps(TF, SF, t_out, s_out)
    res = run_bass_kernel_spmd(nc, in_maps, core_ids=list(range(N_CORES)), trace=_trace)
    out = _assemble(res.results, s_out)
    if _trace:
        return out, res
    return out


# revision 16
# speedup vs baseline: 1.2153x; 1.0487x over previous
"""Trainium2 Bass kernel for nn_Distiller (attention-transfer distillation loss).

Computes on 8 NeuronCores (data-parallel over batch, 2 batches/core):
  SA part: per batch, weighted spatial grams A = V^T V with V = F * sqrt(Fc)
           (Fc = sum |F| over space), for teacher/student features; then
           rho_m = <A_S[m], A_T[m]> / (|A_S[m]| |A_T[m]|) per row.
           Uses the identity sum_n (Ahat_S - Ahat_T)^2 = 2 - 2*rho per row.
  IC part: per batch, channel grams G = L L^T of the [21, 16384] logit maps,
           same rho identity on the 21 rows of G.
Device emits per-row rho partial sums; host assembles the two scalar losses.
s_out passes through on host.
"""

import sys

if "/opt/trn_rl_repo" not in sys.path:
    sys.path.insert(0, "/opt/trn_rl_repo")

import numpy as np
import ml_dtypes

import concourse.bass as bass
import concourse.mybir as mybir
import concourse.tile as tile
from concourse.bass_utils import run_bass_kernel_spmd

# Problem shapes (hardcoded per spec)
B = 16
C = 512
M = 1024  # 32*32 spatial
CC = 21
M2 = 16384  # 128*128 spatial
N_CORES = 8
BPC = B // N_CORES  # batches per core = 2

FP = mybir.dt.float32
FPR = mybir.dt.float32r
BF = mybir.dt.bfloat16
AX = mybir.AxisListType.X
ALU = mybir.AluOpType
ACTF = mybir.ActivationFunctionType



def _split_sync_waits(nc, cap=1):
    """walrus in this container accepts at most `cap` sync waits per
    instruction; hoist excess waits onto same-engine NOPs just before."""
    n = 0
    for f in nc.m.functions:
        for bb in f.blocks:
            newlist = []
            for ins in bb.instructions:
                si = ins.sync_info
                if si is not None and si.on_wait and len(si.on_wait) > cap:
                    waits = list(si.on_wait)
                    hoist, keep = waits[:-cap], waits[-cap:]
                    for w in hoist:
                        n += 1
                        newlist.append(
                            mybir.InstNoOp(
                                name=f"waitsplit-{n}",
                                engine=ins.engine,
                                ins=[],
                                outs=[],
                                sync_info=mybir.SyncInfo(on_wait=[w], on_update=[]),
                            )
                        )
                    si.on_wait = keep
                newlist.append(ins)
            bb.instructions = newlist


def _build():
    nc = bass.Bass(trn_type="TRN2")
    tf = nc.dram_tensor("TF", [BPC, C, M], FP, kind="ExternalInput")
    sf = nc.dram_tensor("SF", [BPC, C, M], FP, kind="ExternalInput")
    # IC inputs: bf16 hi/lo split, both batches packed into 128 rows:
    # rows 0:21 b0-hi, 32:53 b0-lo, 64:85 b1-hi, 96:117 b1-lo (32-aligned so
    # PE-transpose fixups of the hl blocks are tile_position-legal).
    to = nc.dram_tensor("TOHL", [128, M2], BF, kind="ExternalInput")
    so = nc.dram_tensor("SOHL", [128, M2], BF, kind="ExternalInput")
    eye = nc.dram_tensor("EYE", [128, 128], FP, kind="ExternalInput")
    osa = nc.dram_tensor("OSA", [128, 1], FP, kind="ExternalOutput")
    oic = nc.dram_tensor("OIC", [CC, 2], FP, kind="ExternalOutput")

    HH = {0: (0, 0), 1: (64, 64)}
    HL = {0: (0, 32), 1: (64, 96)}

    with tile.TileContext(nc) as tc:
        with (
            tc.tile_pool(name="const", bufs=1) as pconst,
            tc.tile_pool(name="icl", bufs=1) as picl,
            tc.tile_pool(name="vt", bufs=1) as pv,
            tc.tile_pool(name="fstg", bufs=8) as pf,
            tc.tile_pool(name="scr", bufs=3) as pscr,
            tc.tile_pool(name="stat", bufs=1) as pstat,
        ):
            eye_t = pconst.tile([128, 128], FP, name="eye_t")
            nc.sync.dma_start(eye_t[:], eye[:])

            vmats = {}

            def prep_batch(bi):
                # F staged in f32, V = F * sqrt(Fc) written as float32r (the
                # rounding producer the FP32r matmul verifier requires).
                for tname, src in (("t", tf), ("s", sf)):
                    fc = pstat.tile([128, 4], FP, name=f"fc_{tname}_{bi}")
                    sfc = pstat.tile([128, 4], FP, name=f"sfc_{tname}_{bi}")
                    for k in range(C // 128):
                        f = pf.tile([128, M], FP, name=f"fstg_{k}", tag="fstg")
                        nc.sync.dma_start(
                            f[:], src[bi, 128 * k : 128 * (k + 1), :]
                        )
                        nc.vector.tensor_reduce(
                            out=fc[:, k : k + 1],
                            in_=f[:],
                            axis=AX,
                            op=ALU.add,
                            apply_absolute_value=True,
                        )
                        nc.scalar.activation(
                            sfc[:, k : k + 1], fc[:, k : k + 1], ACTF.Sqrt
                        )
                        v = pv.tile([128, M], FPR, name=f"v_{tname}_{bi}_{k}")
                        nc.vector.tensor_scalar_mul(v[:], f[:], sfc[:, k : k + 1])
                        vmats[(tname, bi, k)] = v

            ns2b = pstat.tile([128, 16], FP, name="ns2b")
            nt2b = pstat.tile([128, 16], FP, name="nt2b")
            stb = pstat.tile([128, 16], FP, name="stb")

            def sa_batch(bi, pps):
                for mt in range(8):
                    idx = bi * 8 + mt
                    ps = pps.tile([128, 2048], FP, name="ps", tag="saps")
                    psa = ps[:, 0:M]  # A_T
                    psb = ps[:, M : 2 * M]  # A_S
                    for dst, tname in ((psa, "t"), (psb, "s")):
                        for nh in range(2):
                            for k in range(4):
                                v = vmats[(tname, bi, k)]
                                nc.tensor.matmul(
                                    dst[:, 512 * nh : 512 * (nh + 1)],
                                    lhsT=v[:, 128 * mt : 128 * (mt + 1)],
                                    rhs=v[:, 512 * nh : 512 * (nh + 1)],
                                    start=(k == 0),
                                    stop=(k == 3),
                                )
                    at_sb = pscr.tile([128, M], BF, name="at_sb", tag="atsb")
                    scr1 = pscr.tile([128, M], FP, name="scr1", tag="scr")
                    scr2 = pscr.tile([128, M], FP, name="scr2", tag="scr")
                    scr3 = pscr.tile([128, M], FP, name="scr3", tag="scr")
                    # A_T to SBUF (bf16: DVE 2x copy; SA stats tolerate it)
                    # so each op below touches PSUM at most once.
                    nc.vector.tensor_copy(at_sb[:], psa[:])
                    nc.scalar.activation(
                        scr1[:], psb[:], ACTF.Square,
                        accum_out=ns2b[:, idx : idx + 1],
                    )
                    nc.scalar.activation(
                        scr2[:], psa[:], ACTF.Square,
                        accum_out=nt2b[:, idx : idx + 1],
                    )
                    nc.vector.scalar_tensor_tensor(
                        out=scr3[:],
                        in0=psb[:],
                        scalar=1.0,
                        in1=at_sb[:],
                        op0=ALU.mult,
                        op1=ALU.mult,
                        accum_out=stb[:, idx : idx + 1],
                    )

            # ---- phase order: prep b0, SA b0 | IC | prep b1 (DMA), SA b1.
            # DMA program order on SP: F-b0, IC transposes, F-b1 -> PE is
            # never data-starved.
            prep_batch(0)

            with tc.tile_pool(name="saps0", bufs=2, space="PSUM") as pps0:
                sa_batch(0, pps0)

            # IC loads: hardware DMA-transpose, [128, 4096] chunks land as
            # [128p, 32blk, 128ch] with m = blk*128 + p.
            lmats = {}
            for tname, hl in (("t", to), ("s", so)):
                tt = picl.tile([128, 128, 128], BF, name=f"tt_{tname}")
                for c in range(4):
                    nc.sync.dma_start_transpose(
                        tt[:, 32 * c : 32 * (c + 1), :],
                        hl[:, 4096 * c : 4096 * (c + 1)],
                    )
                lmats[tname] = tt

            prep_batch(1)

            # ---------------- IC phase (own PSUM pools) ----------------
            # One matmul per (tensor, m-block): the [128, 118] product holds
            # hi/lo cross-grams of both batches; G_b = hh + hl + (hl)^T
            # (the lo*lo term is absorbed by row normalization, dropped).
            # Full-width lhsT (128 cols incl zero rows) enables FWL.
            d2 = pstat.tile([128, 2], FP, name="d2")
            with (
                tc.tile_pool(name="icg", bufs=1, space="PSUM") as pg,
                tc.tile_pool(name="icfix", bufs=2, space="PSUM") as pfx,
            ):
                psgs = {}
                for tname in ("t", "s"):
                    tt = lmats[tname]
                    psg = pg.tile([128, 118], FP, name=f"psg_{tname}")
                    for j in range(128):
                        nc.tensor.matmul(
                            psg[:, :],
                            lhsT=tt[:, j, 0:128],
                            rhs=tt[:, j, 0:118],
                            start=(j == 0),
                            stop=(j == 127),
                        )
                    psgs[tname] = psg

                gsbs = {}
                hlts = {}
                for tname in ("t", "s"):
                    gsb = pstat.tile([128, 118], FP, name=f"gsb_{tname}")
                    nc.scalar.copy(gsb[:], psgs[tname][:])
                    gsbs[tname] = gsb
                    # fix-up: lh contribution = (hl)^T; transpose the hl block
                    # (rows at base 0/64 are tile_position-legal inputs). Each
                    # transpose gets its own PSUM tile (same-tile column-
                    # disjoint transpose outputs fault at runtime); b1's block
                    # is realigned to partition 64 via an identity matmul.
                    hlt = pstat.tile([128, 21], FP, name=f"hlt_{tname}")
                    pstr0 = pfx.tile([128, 21], FP, name=f"pstr0_{tname}", tag="fx0")
                    rp, cp = HL[0]
                    nc.tensor.transpose(
                        pstr0[0:21, 0:21],
                        gsb[rp : rp + 21, cp : cp + 21],
                        eye_t[rp : rp + 21, rp : rp + 21],
                    )
                    nc.scalar.copy(hlt[0:21, :], pstr0[0:21, :])
                    pstr1 = pfx.tile([128, 21], FP, name=f"pstr1_{tname}", tag="fx1")
                    rp, cp = HL[1]
                    nc.tensor.transpose(
                        pstr1[0:21, 0:21],
                        gsb[rp : rp + 21, cp : cp + 21],
                        eye_t[rp : rp + 21, rp : rp + 21],
                    )
                    h1sb = pstat.tile([128, 21], FP, name=f"h1sb_{tname}")
                    nc.scalar.copy(h1sb[0:21, :], pstr1[0:21, :])
                    ps2 = pfx.tile([128, 21], FP, name=f"ps2_{tname}", tag="fx2")
                    nc.tensor.matmul(
                        ps2[64:85, :],
                        lhsT=eye_t[0:21, 0:21],
                        rhs=h1sb[0:21, 0:21],
                        start=True,
                        stop=True,
                    )
                    nc.scalar.copy(hlt[64:85, :], ps2[64:85, :])
                    hlts[tname] = hlt

                # G_b = hh + hl + hlT at partition base 64*bi; then the
                # cancellation-free loss form: D = Gs/|Gs| - Gt/|Gt| rows,
                # d2 = rowsum(D^2). (rho form would amplify rounding ~800x.)
                gsum = {}
                for tname in ("t", "s"):
                    g = pstat.tile([128, 21], FP, name=f"gsum_{tname}")
                    for bi in range(BPC):
                        rp = HH[bi][0]
                        hh = gsbs[tname][rp : rp + 21, HH[bi][1] : HH[bi][1] + 21]
                        hlc = gsbs[tname][rp : rp + 21, HL[bi][1] : HL[bi][1] + 21]
                        hlt = hlts[tname][rp : rp + 21, 0:21]
                        gslice = g[rp : rp + 21, :]
                        nc.vector.tensor_tensor(gslice, hh, hlc, op=ALU.add)
                        nc.vector.tensor_tensor(gslice, gslice, hlt, op=ALU.add)
                    gsum[tname] = g
                rs = pstat.tile([128, 2], FP, name="rs")
                rt = pstat.tile([128, 2], FP, name="rt")
                scr_ic = pstat.tile([128, 21], FP, name="scr_ic")
                scr_ic2 = pstat.tile([128, 21], FP, name="scr_ic2")
                scr_ic3 = pstat.tile([128, 21], FP, name="scr_ic3")
                for bi in range(BPC):
                    o = HH[bi][0]
                    gs = gsum["s"][o : o + 21, :]
                    gt = gsum["t"][o : o + 21, :]
                    # rs = 1/|Gs row|, rt = 1/|Gt row|
                    nc.scalar.activation(
                        scr_ic[o : o + 21, :], gs, ACTF.Square,
                        accum_out=rs[o : o + 21, bi : bi + 1],
                    )
                    nc.scalar.activation(
                        scr_ic[o : o + 21, :], gt, ACTF.Square,
                        accum_out=rt[o : o + 21, bi : bi + 1],
                    )
                    nc.scalar.activation(
                        rs[o : o + 21, bi : bi + 1],
                        rs[o : o + 21, bi : bi + 1], ACTF.Sqrt,
                    )
                    nc.scalar.activation(
                        rt[o : o + 21, bi : bi + 1],
                        rt[o : o + 21, bi : bi + 1], ACTF.Sqrt,
                    )
                    nc.vector.reciprocal(
                        rs[o : o + 21, bi : bi + 1], rs[o : o + 21, bi : bi + 1]
                    )
                    nc.vector.reciprocal(
                        rt[o : o + 21, bi : bi + 1], rt[o : o + 21, bi : bi + 1]
                    )
                    # X = Gt*rt ; D = Gs*rs - X ; d2 = rowsum(D^2)
                    nc.vector.tensor_scalar_mul(
                        scr_ic2[o : o + 21, :], gt, rt[o : o + 21, bi : bi + 1]
                    )
                    nc.vector.scalar_tensor_tensor(
                        out=scr_ic3[o : o + 21, :],
                        in0=gs,
                        scalar=rs[o : o + 21, bi : bi + 1],
                        in1=scr_ic2[o : o + 21, :],
                        op0=ALU.mult,
                        op1=ALU.subtract,
                    )
                    nc.scalar.activation(
                        scr_ic2[o : o + 21, :],
                        scr_ic3[o : o + 21, :],
                        ACTF.Square,
                        accum_out=d2[o : o + 21, bi : bi + 1],
                    )
            nc.sync.dma_start(oic[:, 0:1], d2[0:21, 0:1])
            nc.sync.dma_start(oic[:, 1:2], d2[64:85, 1:2])

            # ---------------- SA batch 1 ----------------
            with tc.tile_pool(name="saps1", bufs=2, space="PSUM") as pps1:
                sa_batch(1, pps1)

            # final SA math: rho rows -> per-partition sums -> OSA
            rn = pstat.tile([128, 16], FP, name="rn")
            nc.vector.tensor_tensor(rn[:], ns2b[:], nt2b[:], op=ALU.mult)
            nc.scalar.activation(rn[:], rn[:], ACTF.Sqrt)
            nc.vector.reciprocal(rn[:], rn[:])
            prho = pstat.tile([128, 16], FP, name="prho")
            nc.vector.tensor_tensor(prho[:], stb[:], rn[:], op=ALU.mult)
            osa_t = pstat.tile([128, 1], FP, name="osa_t")
            nc.vector.tensor_reduce(
                out=osa_t[:], in_=prho[:], axis=AX, op=ALU.add
            )
            nc.sync.dma_start(osa[:], osa_t[:])

    _split_sync_waits(nc)
    return nc


_NC = None


def _get_nc():
    global _NC
    if _NC is None:
        _NC = _build()
    return _NC


_EYE = np.eye(128, dtype=np.float32)


def _hl_pack(X):
    """[B, CC, M2] f32 -> per-core [128, M2] bf16 with rows
    0:21 b0-hi, 32:53 b0-lo, 64:85 b1-hi, 96:117 b1-lo."""
    bf = ml_dtypes.bfloat16
    hi = X.astype(bf)
    lo = (X - hi.astype(np.float32)).astype(bf)
    out = np.zeros((N_CORES, 128, M2), dtype=bf)
    out[:, 0:21] = hi[0::2]
    out[:, 32:53] = lo[0::2]
    out[:, 64:85] = hi[1::2]
    out[:, 96:117] = lo[1::2]
    return out


def _make_in_maps(TF, SF, t_out, s_out):
    TFr = np.ascontiguousarray(TF.reshape(B, C, M), dtype=np.float32)
    SFr = np.ascontiguousarray(SF.reshape(B, C, M), dtype=np.float32)
    TOhl = _hl_pack(np.asarray(t_out, dtype=np.float32).reshape(B, CC, M2))
    SOhl = _hl_pack(np.asarray(s_out, dtype=np.float32).reshape(B, CC, M2))
    in_maps = []
    for i in range(N_CORES):
        sl = slice(BPC * i, BPC * (i + 1))
        in_maps.append(
            {
                "TF": np.ascontiguousarray(TFr[sl]),
                "SF": np.ascontiguousarray(SFr[sl]),
                "TOHL": np.ascontiguousarray(TOhl[i]),
                "SOHL": np.ascontiguousarray(SOhl[i]),
                "EYE": _EYE,
            }
        )
    return in_maps


def _assemble(results, s_out):
    sa_rho = 0.0
    ic_rho = 0.0
    for r in results:
        sa_rho += float(r["OSA"].astype(np.float64).sum())
        ic_rho += float(r["OIC"].astype(np.float64).sum())
    sa_loss = (2.0 * B * M - 2.0 * sa_rho) / (B * M * M)
    ic_loss = ic_rho / (CC * B)  # OIC carries rowsum((Ghat_s - Ghat_t)^2)
    return (s_out, np.float32(ic_loss), np.float32(sa_loss))


def kernel(TF, SF, t_out, s_out, _trace=False):
    nc = _get_nc()
    in_maps = _make_in_maps(TF, SF, t_out, s_out)
    res = run_bass_kernel_spmd(nc, in_maps, core_ids=list(range(N_CORES)), trace=_trace)
    out = _assemble(res.results, s_out)
    if _trace:
        return out, res
    return out


# revision 17
# speedup vs baseline: 1.3744x; 1.1310x over previous
"""Trainium2 Bass kernel for nn_Distiller (attention-transfer distillation loss).

Computes on 8 NeuronCores (data-parallel over batch, 2 batches/core):
  SA part: per batch, weighted spatial grams A = V^T V with V = F * sqrt(Fc)
           (Fc = sum |F| over space), for teacher/student features; then
           rho_m = <A_S[m], A_T[m]> / (|A_S[m]| |A_T[m]|) per row.
           Uses the identity sum_n (Ahat_S - Ahat_T)^2 = 2 - 2*rho per row.
  IC part: per batch, channel grams G = L L^T of the [21, 16384] logit maps,
           same rho identity on the 21 rows of G.
Device emits per-row rho partial sums; host assembles the two scalar losses.
s_out passes through on host.
"""

import sys

if "/opt/trn_rl_repo" not in sys.path:
    sys.path.insert(0, "/opt/trn_rl_repo")

import numpy as np
import ml_dtypes

import concourse.bass as bass
import concourse.mybir as mybir
import concourse.tile as tile
from concourse.bass_utils import run_bass_kernel_spmd

# Problem shapes (hardcoded per spec)
B = 16
C = 512
M = 1024  # 32*32 spatial
CC = 21
M2 = 16384  # 128*128 spatial
N_CORES = 8
BPC = B // N_CORES  # batches per core = 2

FP = mybir.dt.float32
FPR = mybir.dt.float32r
BF = mybir.dt.bfloat16
AX = mybir.AxisListType.X
ALU = mybir.AluOpType
ACTF = mybir.ActivationFunctionType



def _split_sync_waits(nc, cap=1):
    """walrus in this container accepts at most `cap` sync waits per
    instruction; hoist excess waits onto same-engine NOPs just before."""
    n = 0
    for f in nc.m.functions:
        for bb in f.blocks:
            newlist = []
            for ins in bb.instructions:
                si = ins.sync_info
                if si is not None and si.on_wait and len(si.on_wait) > cap:
                    waits = list(si.on_wait)
                    hoist, keep = waits[:-cap], waits[-cap:]
                    for w in hoist:
                        n += 1
                        newlist.append(
                            mybir.InstNoOp(
                                name=f"waitsplit-{n}",
                                engine=ins.engine,
                                ins=[],
                                outs=[],
                                sync_info=mybir.SyncInfo(on_wait=[w], on_update=[]),
                            )
                        )
                    si.on_wait = keep
                newlist.append(ins)
            bb.instructions = newlist


def _build():
    nc = bass.Bass(trn_type="TRN2")
    tf = nc.dram_tensor("TF", [BPC, C, M], FP, kind="ExternalInput")
    sf = nc.dram_tensor("SF", [BPC, C, M], FP, kind="ExternalInput")
    # IC inputs: bf16 hi/lo split, both batches packed into 128 rows:
    # rows 0:21 b0-hi, 32:53 b0-lo, 64:85 b1-hi, 96:117 b1-lo (32-aligned so
    # PE-transpose fixups of the hl blocks are tile_position-legal).
    to = nc.dram_tensor("TOHL", [128, M2], BF, kind="ExternalInput")
    so = nc.dram_tensor("SOHL", [128, M2], BF, kind="ExternalInput")
    eye = nc.dram_tensor("EYE", [128, 128], FP, kind="ExternalInput")
    osa = nc.dram_tensor("OSA", [128, 1], FP, kind="ExternalOutput")
    oic = nc.dram_tensor("OIC", [CC, 2], FP, kind="ExternalOutput")

    HH = {0: (0, 0), 1: (64, 64)}
    HL = {0: (0, 32), 1: (64, 96)}

    with tile.TileContext(nc) as tc:
        with (
            tc.tile_pool(name="const", bufs=1) as pconst,
            tc.tile_pool(name="icl", bufs=1) as picl,
            tc.tile_pool(name="vt", bufs=1) as pv,
            tc.tile_pool(name="fstg", bufs=8) as pf,
            tc.tile_pool(name="scr", bufs=3) as pscr,
            tc.tile_pool(name="stat", bufs=1) as pstat,
        ):
            eye_t = pconst.tile([128, 128], FP, name="eye_t")
            nc.sync.dma_start(eye_t[:], eye[:])

            vmats = {}

            def prep_batch(bi):
                # F staged in f32, V = F * sqrt(Fc) written as float32r (the
                # rounding producer the FP32r matmul verifier requires).
                for tname, src in (("t", tf), ("s", sf)):
                    fc = pstat.tile([128, 4], FP, name=f"fc_{tname}_{bi}")
                    sfc = pstat.tile([128, 4], FP, name=f"sfc_{tname}_{bi}")
                    for k in range(C // 128):
                        f = pf.tile([128, M], FP, name=f"fstg_{k}", tag="fstg")
                        nc.sync.dma_start(
                            f[:], src[bi, 128 * k : 128 * (k + 1), :]
                        )
                        nc.vector.tensor_reduce(
                            out=fc[:, k : k + 1],
                            in_=f[:],
                            axis=AX,
                            op=ALU.add,
                            apply_absolute_value=True,
                        )
                        nc.scalar.activation(
                            sfc[:, k : k + 1], fc[:, k : k + 1], ACTF.Sqrt
                        )
                        v = pv.tile([128, M], FPR, name=f"v_{tname}_{bi}_{k}")
                        nc.vector.tensor_scalar_mul(v[:], f[:], sfc[:, k : k + 1])
                        vmats[(tname, bi, k)] = v

            ns2b = pstat.tile([128, 16], FP, name="ns2b")
            nt2b = pstat.tile([128, 16], FP, name="nt2b")
            stb = pstat.tile([128, 16], FP, name="stb")

            def sa_batch(bi, pps):
                for mt in range(8):
                    idx = bi * 8 + mt
                    # separate PSUM tiles per gram: consumers of A_T start as
                    # soon as its 8 matmuls finish (one shared tile would make
                    # every reader wait for all 16).
                    psa = pps.tile([128, M], FP, name="psa", tag="psa")  # A_T
                    psb = pps.tile([128, M], FP, name="psb", tag="psb")  # A_S
                    for dst, tname in ((psa, "t"), (psb, "s")):
                        for nh in range(2):
                            for k in range(4):
                                v = vmats[(tname, bi, k)]
                                nc.tensor.matmul(
                                    dst[:, 512 * nh : 512 * (nh + 1)],
                                    lhsT=v[:, 128 * mt : 128 * (mt + 1)],
                                    rhs=v[:, 512 * nh : 512 * (nh + 1)],
                                    start=(k == 0),
                                    stop=(k == 3),
                                )
                    at_sb = pscr.tile([128, M], BF, name="at_sb", tag="atsb")
                    scr1 = pscr.tile([128, M], FP, name="scr1", tag="scr")
                    scr2 = pscr.tile([128, M], FP, name="scr2", tag="scr")
                    scr3 = pscr.tile([128, M], FP, name="scr3", tag="scr")
                    # A_T to SBUF (bf16: DVE 2x copy; SA stats tolerate it)
                    # so each op below touches PSUM at most once.
                    nc.vector.tensor_copy(at_sb[:], psa[:])
                    nc.scalar.activation(
                        scr2[:], psa[:], ACTF.Square,
                        accum_out=nt2b[:, idx : idx + 1],
                    )
                    nc.scalar.activation(
                        scr1[:], psb[:], ACTF.Square,
                        accum_out=ns2b[:, idx : idx + 1],
                    )
                    nc.vector.scalar_tensor_tensor(
                        out=scr3[:],
                        in0=psb[:],
                        scalar=1.0,
                        in1=at_sb[:],
                        op0=ALU.mult,
                        op1=ALU.mult,
                        accum_out=stb[:, idx : idx + 1],
                    )

            # ---- phase order: prep b0, SA b0 | IC | prep b1 (DMA), SA b1.
            # DMA program order on SP: F-b0, IC transposes, F-b1 -> PE is
            # never data-starved.
            prep_batch(0)

            with tc.tile_pool(name="saps0", bufs=2, space="PSUM") as pps0:
                sa_batch(0, pps0)

            # IC loads: hardware DMA-transpose, [128, 4096] chunks land as
            # [128p, 32blk, 128ch] with m = blk*128 + p.
            lmats = {}
            for tname, hl in (("t", to), ("s", so)):
                tt = picl.tile([128, 128, 128], BF, name=f"tt_{tname}")
                for c in range(4):
                    nc.sync.dma_start_transpose(
                        tt[:, 32 * c : 32 * (c + 1), :],
                        hl[:, 4096 * c : 4096 * (c + 1)],
                    )
                lmats[tname] = tt

            prep_batch(1)

            # ---------------- IC phase (own PSUM pools) ----------------
            # One matmul per (tensor, m-block): the [128, 118] product holds
            # hi/lo cross-grams of both batches; G_b = hh + hl + (hl)^T
            # (the lo*lo term is absorbed by row normalization, dropped).
            # Full-width lhsT (128 cols incl zero rows) enables FWL.
            d2 = pstat.tile([128, 2], FP, name="d2")
            with (
                tc.tile_pool(name="icg", bufs=1, space="PSUM") as pg,
                tc.tile_pool(name="icfix", bufs=2, space="PSUM") as pfx,
            ):
                psgs = {}
                for tname in ("t", "s"):
                    tt = lmats[tname]
                    psg = pg.tile([128, 118], FP, name=f"psg_{tname}")
                    for j in range(128):
                        nc.tensor.matmul(
                            psg[:, :],
                            lhsT=tt[:, j, 0:128],
                            rhs=tt[:, j, 0:118],
                            start=(j == 0),
                            stop=(j == 127),
                        )
                    psgs[tname] = psg

                gsbs = {}
                hlts = {}
                for tname in ("t", "s"):
                    gsb = pstat.tile([128, 118], FP, name=f"gsb_{tname}")
                    nc.scalar.copy(gsb[:], psgs[tname][:])
                    gsbs[tname] = gsb
                    # fix-up: lh contribution = (hl)^T; transpose the hl block
                    # (rows at base 0/64 are tile_position-legal inputs). Each
                    # transpose gets its own PSUM tile (same-tile column-
                    # disjoint transpose outputs fault at runtime); b1's block
                    # is realigned to partition 64 via an identity matmul.
                    hlt = pstat.tile([128, 21], FP, name=f"hlt_{tname}")
                    pstr0 = pfx.tile([128, 21], FP, name=f"pstr0_{tname}", tag="fx0")
                    rp, cp = HL[0]
                    nc.tensor.transpose(
                        pstr0[0:21, 0:21],
                        gsb[rp : rp + 21, cp : cp + 21],
                        eye_t[rp : rp + 21, rp : rp + 21],
                    )
                    nc.scalar.copy(hlt[0:21, :], pstr0[0:21, :])
                    pstr1 = pfx.tile([128, 21], FP, name=f"pstr1_{tname}", tag="fx1")
                    rp, cp = HL[1]
                    nc.tensor.transpose(
                        pstr1[0:21, 0:21],
                        gsb[rp : rp + 21, cp : cp + 21],
                        eye_t[rp : rp + 21, rp : rp + 21],
                    )
                    h1sb = pstat.tile([128, 21], FP, name=f"h1sb_{tname}")
                    nc.scalar.copy(h1sb[0:21, :], pstr1[0:21, :])
                    ps2 = pfx.tile([128, 21], FP, name=f"ps2_{tname}", tag="fx2")
                    nc.tensor.matmul(
                        ps2[64:85, :],
                        lhsT=eye_t[0:21, 0:21],
                        rhs=h1sb[0:21, 0:21],
                        start=True,
                        stop=True,
                    )
                    nc.scalar.copy(hlt[64:85, :], ps2[64:85, :])
                    hlts[tname] = hlt

                # G_b = hh + hl + hlT at partition base 64*bi; then the
                # cancellation-free loss form: D = Gs/|Gs| - Gt/|Gt| rows,
                # d2 = rowsum(D^2). (rho form would amplify rounding ~800x.)
                gsum = {}
                for tname in ("t", "s"):
                    g = pstat.tile([128, 21], FP, name=f"gsum_{tname}")
                    for bi in range(BPC):
                        rp = HH[bi][0]
                        hh = gsbs[tname][rp : rp + 21, HH[bi][1] : HH[bi][1] + 21]
                        hlc = gsbs[tname][rp : rp + 21, HL[bi][1] : HL[bi][1] + 21]
                        hlt = hlts[tname][rp : rp + 21, 0:21]
                        gslice = g[rp : rp + 21, :]
                        nc.vector.tensor_tensor(gslice, hh, hlc, op=ALU.add)
                        nc.vector.tensor_tensor(gslice, gslice, hlt, op=ALU.add)
                    gsum[tname] = g
                rs = pstat.tile([128, 2], FP, name="rs")
                rt = pstat.tile([128, 2], FP, name="rt")
                scr_ic = pstat.tile([128, 21], FP, name="scr_ic")
                scr_ic2 = pstat.tile([128, 21], FP, name="scr_ic2")
                scr_ic3 = pstat.tile([128, 21], FP, name="scr_ic3")
                for bi in range(BPC):
                    o = HH[bi][0]
                    gs = gsum["s"][o : o + 21, :]
                    gt = gsum["t"][o : o + 21, :]
                    # rs = 1/|Gs row|, rt = 1/|Gt row|
                    nc.scalar.activation(
                        scr_ic[o : o + 21, :], gs, ACTF.Square,
                        accum_out=rs[o : o + 21, bi : bi + 1],
                    )
                    nc.scalar.activation(
                        scr_ic[o : o + 21, :], gt, ACTF.Square,
                        accum_out=rt[o : o + 21, bi : bi + 1],
                    )
                    nc.scalar.activation(
                        rs[o : o + 21, bi : bi + 1],
                        rs[o : o + 21, bi : bi + 1], ACTF.Sqrt,
                    )
                    nc.scalar.activation(
                        rt[o : o + 21, bi : bi + 1],
                        rt[o : o + 21, bi : bi + 1], ACTF.Sqrt,
                    )
                    nc.vector.reciprocal(
                        rs[o : o + 21, bi : bi + 1], rs[o : o + 21, bi : bi + 1]
                    )
                    nc.vector.reciprocal(
                        rt[o : o + 21, bi : bi + 1], rt[o : o + 21, bi : bi + 1]
                    )
                    # X = Gt*rt ; D = Gs*rs - X ; d2 = rowsum(D^2)
                    nc.vector.tensor_scalar_mul(
                        scr_ic2[o : o + 21, :], gt, rt[o : o + 21, bi : bi + 1]
                    )
                    nc.vector.scalar_tensor_tensor(
                        out=scr_ic3[o : o + 21, :],
                        in0=gs,
                        scalar=rs[o : o + 21, bi : bi + 1],
                        in1=scr_ic2[o : o + 21, :],
                        op0=ALU.mult,
                        op1=ALU.subtract,
                    )
                    nc.scalar.activation(
                        scr_ic2[o : o + 21, :],
                        scr_ic3[o : o + 21, :],
                        ACTF.Square,
                        accum_out=d2[o : o + 21, bi : bi + 1],
                    )
            nc.sync.dma_start(oic[:, 0:1], d2[0:21, 0:1])
            nc.sync.dma_start(oic[:, 1:2], d2[64:85, 1:2])

            # ---------------- SA batch 1 ----------------
            with tc.tile_pool(name="saps1", bufs=2, space="PSUM") as pps1:
                sa_batch(1, pps1)

            # final SA math: rho rows -> per-partition sums -> OSA
            rn = pstat.tile([128, 16], FP, name="rn")
            nc.vector.tensor_tensor(rn[:], ns2b[:], nt2b[:], op=ALU.mult)
            nc.scalar.activation(rn[:], rn[:], ACTF.Sqrt)
            nc.vector.reciprocal(rn[:], rn[:])
            prho = pstat.tile([128, 16], FP, name="prho")
            nc.vector.tensor_tensor(prho[:], stb[:], rn[:], op=ALU.mult)
            osa_t = pstat.tile([128, 1], FP, name="osa_t")
            nc.vector.tensor_reduce(
                out=osa_t[:], in_=prho[:], axis=AX, op=ALU.add
            )
            nc.sync.dma_start(osa[:], osa_t[:])

    _split_sync_waits(nc)
    return nc


_NC = None


def _get_nc():
    global _NC
    if _NC is None:
        _NC = _build()
    return _NC


_EYE = np.eye(128, dtype=np.float32)


def _hl_pack(X):
    """[B, CC, M2] f32 -> per-core [128, M2] bf16 with rows
    0:21 b0-hi, 32:53 b0-lo, 64:85 b1-hi, 96:117 b1-lo."""
    bf = ml_dtypes.bfloat16
    hi = X.astype(bf)
    lo = (X - hi.astype(np.float32)).astype(bf)
    out = np.zeros((N_CORES, 128, M2), dtype=bf)
    out[:, 0:21] = hi[0::2]
    out[:, 32:53] = lo[0::2]
    out[:, 64:85] = hi[1::2]
    out[:, 96:117] = lo[1::2]
    return out


def _make_in_maps(TF, SF, t_out, s_out):
    TFr = np.ascontiguousarray(TF.reshape(B, C, M), dtype=np.float32)
    SFr = np.ascontiguousarray(SF.reshape(B, C, M), dtype=np.float32)
    TOhl = _hl_pack(np.asarray(t_out, dtype=np.float32).reshape(B, CC, M2))
    SOhl = _hl_pack(np.asarray(s_out, dtype=np.float32).reshape(B, CC, M2))
    in_maps = []
    for i in range(N_CORES):
        sl = slice(BPC * i, BPC * (i + 1))
        in_maps.append(
            {
                "TF": np.ascontiguousarray(TFr[sl]),
                "SF": np.ascontiguousarray(SFr[sl]),
                "TOHL": np.ascontiguousarray(TOhl[i]),
                "SOHL": np.ascontiguousarray(SOhl[i]),
                "EYE": _EYE,
            }
        )
    return in_maps


def _assemble(results, s_out):
    sa_rho = 0.0
    ic_rho = 0.0
    for r in results:
        sa_rho += float(r["OSA"].astype(np.float64).sum())
        ic_rho += float(r["OIC"].astype(np.float64).sum())
    sa_loss = (2.0 * B * M - 2.0 * sa_rho) / (B * M * M)
    ic_loss = ic_rho / (CC * B)  # OIC carries rowsum((Ghat_s - Ghat_t)^2)
    return (s_out, np.float32(ic_loss), np.float32(sa_loss))


def kernel(TF, SF, t_out, s_out, _trace=False):
    nc = _get_nc()
    in_maps = _make_in_maps(TF, SF, t_out, s_out)
    res = run_bass_kernel_spmd(nc, in_maps, core_ids=list(range(N_CORES)), trace=_trace)
    out = _assemble(res.results, s_out)
    if _trace:
        return out, res
    return out


# revision 18
# speedup vs baseline: 1.6018x; 1.1655x over previous
"""Trainium2 Bass kernel for nn_Distiller (attention-transfer distillation loss).

Computes on 8 NeuronCores (data-parallel over batch, 2 batches/core):
  SA part: per batch, weighted spatial grams A = V^T V with V = F * sqrt(Fc)
           (Fc = sum |F| over space), for teacher/student features; then
           rho_m = <A_S[m], A_T[m]> / (|A_S[m]| |A_T[m]|) per row.
           Uses the identity sum_n (Ahat_S - Ahat_T)^2 = 2 - 2*rho per row.
  IC part: per batch, channel grams G = L L^T of the [21, 16384] logit maps,
           same rho identity on the 21 rows of G.
Device emits per-row rho partial sums; host assembles the two scalar losses.
s_out passes through on host.
"""

import sys

if "/opt/trn_rl_repo" not in sys.path:
    sys.path.insert(0, "/opt/trn_rl_repo")

import numpy as np
import ml_dtypes

import concourse.bass as bass
import concourse.mybir as mybir
import concourse.tile as tile
from concourse.bass_utils import run_bass_kernel_spmd

# Problem shapes (hardcoded per spec)
B = 16
C = 512
M = 1024  # 32*32 spatial
CC = 21
M2 = 16384  # 128*128 spatial
N_CORES = 8
BPC = B // N_CORES  # batches per core = 2

FP = mybir.dt.float32
FPR = mybir.dt.float32r
BF = mybir.dt.bfloat16
AX = mybir.AxisListType.X
ALU = mybir.AluOpType
ACTF = mybir.ActivationFunctionType



def _split_sync_waits(nc, cap=1):
    """walrus in this container accepts at most `cap` sync waits per
    instruction; hoist excess waits onto same-engine NOPs just before."""
    n = 0
    for f in nc.m.functions:
        for bb in f.blocks:
            newlist = []
            for ins in bb.instructions:
                si = ins.sync_info
                if si is not None and si.on_wait and len(si.on_wait) > cap:
                    waits = list(si.on_wait)
                    hoist, keep = waits[:-cap], waits[-cap:]
                    for w in hoist:
                        n += 1
                        newlist.append(
                            mybir.InstNoOp(
                                name=f"waitsplit-{n}",
                                engine=ins.engine,
                                ins=[],
                                outs=[],
                                sync_info=mybir.SyncInfo(on_wait=[w], on_update=[]),
                            )
                        )
                    si.on_wait = keep
                newlist.append(ins)
            bb.instructions = newlist


def _build():
    nc = bass.Bass(trn_type="TRN2")
    tf = nc.dram_tensor("TF", [BPC, C, M], FP, kind="ExternalInput")
    sf = nc.dram_tensor("SF", [BPC, C, M], FP, kind="ExternalInput")
    # IC inputs: bf16 hi/lo split, both batches packed into 128 rows:
    # rows 0:21 b0-hi, 32:53 b0-lo, 64:85 b1-hi, 96:117 b1-lo (32-aligned so
    # PE-transpose fixups of the hl blocks are tile_position-legal).
    to = nc.dram_tensor("TOHL", [128, M2], BF, kind="ExternalInput")
    so = nc.dram_tensor("SOHL", [128, M2], BF, kind="ExternalInput")
    ost = nc.dram_tensor("OST", [3, 128, 16], FP, kind="ExternalOutput")
    og = nc.dram_tensor("OG", [2, 128, 118], FP, kind="ExternalOutput")

    with tile.TileContext(nc) as tc:
        with (
            tc.tile_pool(name="icl", bufs=1) as picl,
            tc.tile_pool(name="vt", bufs=1) as pv,
            tc.tile_pool(name="fstg", bufs=8) as pf,
            tc.tile_pool(name="scr", bufs=3) as pscr,
            tc.tile_pool(name="stat", bufs=1) as pstat,
        ):
            vmats = {}

            def prep_batch(bi):
                # F staged in f32, V = F * sqrt(Fc) written as float32r (the
                # rounding producer the FP32r matmul verifier requires).
                for tname, src in (("t", tf), ("s", sf)):
                    fc = pstat.tile([128, 4], FP, name=f"fc_{tname}_{bi}")
                    sfc = pstat.tile([128, 4], FP, name=f"sfc_{tname}_{bi}")
                    for k in range(C // 128):
                        f = pf.tile([128, M], FP, name=f"fstg_{k}", tag="fstg")
                        nc.sync.dma_start(
                            f[:], src[bi, 128 * k : 128 * (k + 1), :]
                        )
                        nc.vector.tensor_reduce(
                            out=fc[:, k : k + 1],
                            in_=f[:],
                            axis=AX,
                            op=ALU.add,
                            apply_absolute_value=True,
                        )
                        nc.scalar.activation(
                            sfc[:, k : k + 1], fc[:, k : k + 1], ACTF.Sqrt
                        )
                        v = pv.tile([128, M], FPR, name=f"v_{tname}_{bi}_{k}")
                        nc.vector.tensor_scalar_mul(v[:], f[:], sfc[:, k : k + 1])
                        vmats[(tname, bi, k)] = v

            ns2b = pstat.tile([128, 16], FP, name="ns2b")
            nt2b = pstat.tile([128, 16], FP, name="nt2b")
            stb = pstat.tile([128, 16], FP, name="stb")

            def sa_batch(bi, pps):
                for mt in range(8):
                    idx = bi * 8 + mt
                    # separate PSUM tiles per gram: consumers of A_T start as
                    # soon as its 8 matmuls finish (one shared tile would make
                    # every reader wait for all 16).
                    psa = pps.tile([128, M], FP, name="psa", tag="psa")  # A_T
                    psb = pps.tile([128, M], FP, name="psb", tag="psb")  # A_S
                    for dst, tname in ((psa, "t"), (psb, "s")):
                        for nh in range(2):
                            for k in range(4):
                                v = vmats[(tname, bi, k)]
                                nc.tensor.matmul(
                                    dst[:, 512 * nh : 512 * (nh + 1)],
                                    lhsT=v[:, 128 * mt : 128 * (mt + 1)],
                                    rhs=v[:, 512 * nh : 512 * (nh + 1)],
                                    start=(k == 0),
                                    stop=(k == 3),
                                )
                    at_sb = pscr.tile([128, M], BF, name="at_sb", tag="atsb")
                    scr1 = pscr.tile([128, M], FP, name="scr1", tag="scr")
                    scr2 = pscr.tile([128, M], FP, name="scr2", tag="scr")
                    scr3 = pscr.tile([128, M], FP, name="scr3", tag="scr")
                    # A_T to SBUF (bf16: DVE 2x copy; SA stats tolerate it)
                    # so each op below touches PSUM at most once.
                    nc.vector.tensor_copy(at_sb[:], psa[:])
                    nc.scalar.activation(
                        scr2[:], psa[:], ACTF.Square,
                        accum_out=nt2b[:, idx : idx + 1],
                    )
                    nc.scalar.activation(
                        scr1[:], psb[:], ACTF.Square,
                        accum_out=ns2b[:, idx : idx + 1],
                    )
                    nc.vector.scalar_tensor_tensor(
                        out=scr3[:],
                        in0=psb[:],
                        scalar=1.0,
                        in1=at_sb[:],
                        op0=ALU.mult,
                        op1=ALU.mult,
                        accum_out=stb[:, idx : idx + 1],
                    )

            # ---- phase order: prep b0, SA b0 | IC | prep b1 (DMA), SA b1.
            # DMA program order on SP: F-b0, IC transposes, F-b1 -> PE is
            # never data-starved.
            prep_batch(0)

            with tc.tile_pool(name="saps0", bufs=2, space="PSUM") as pps0:
                sa_batch(0, pps0)

            # IC loads: hardware DMA-transpose, [128, 4096] chunks land as
            # [128p, 32blk, 128ch] with m = blk*128 + p.
            lmats = {}
            for tname, hl in (("t", to), ("s", so)):
                tt = picl.tile([128, 128, 128], BF, name=f"tt_{tname}")
                for c in range(4):
                    nc.sync.dma_start_transpose(
                        tt[:, 32 * c : 32 * (c + 1), :],
                        hl[:, 4096 * c : 4096 * (c + 1)],
                    )
                lmats[tname] = tt

            prep_batch(1)

            # ---------------- IC phase (own PSUM pool) ----------------
            # One matmul per (tensor, m-block): the [128, 118] product holds
            # hi/lo cross-grams of both batches. The [21,21] block extraction
            # G = hh + hl + hl^T + ll and the loss math happen on the host
            # (f64) from the shipped OG blocks.
            with tc.tile_pool(name="icg", bufs=1, space="PSUM") as pg:
                for ti, tname in enumerate(("t", "s")):
                    tt = lmats[tname]
                    psg = pg.tile([128, 118], FP, name=f"psg_{tname}")
                    for j in range(128):
                        nc.tensor.matmul(
                            psg[:, :],
                            lhsT=tt[:, j, 0:128],
                            rhs=tt[:, j, 0:118],
                            start=(j == 0),
                            stop=(j == 127),
                        )
                    gsb = pstat.tile([128, 118], FP, name=f"gsb_{tname}")
                    nc.scalar.copy(gsb[:], psg[:])
                    nc.sync.dma_start(og[ti], gsb[:])

            # ---------------- SA batch 1 ----------------
            with tc.tile_pool(name="saps1", bufs=2, space="PSUM") as pps1:
                sa_batch(1, pps1)

            # ship raw SA stats; host does rho math in f64
            nc.sync.dma_start(ost[0], stb[:])
            nc.sync.dma_start(ost[1], ns2b[:])
            nc.sync.dma_start(ost[2], nt2b[:])

    _split_sync_waits(nc)
    return nc


_NC = None


def _get_nc():
    global _NC
    if _NC is None:
        _NC = _build()
    return _NC


def _hl_pack(X):
    """[B, CC, M2] f32 -> per-core [128, M2] bf16 with rows
    0:21 b0-hi, 32:53 b0-lo, 64:85 b1-hi, 96:117 b1-lo."""
    bf = ml_dtypes.bfloat16
    hi = X.astype(bf)
    lo = (X - hi.astype(np.float32)).astype(bf)
    out = np.zeros((N_CORES, 128, M2), dtype=bf)
    out[:, 0:21] = hi[0::2]
    out[:, 32:53] = lo[0::2]
    out[:, 64:85] = hi[1::2]
    out[:, 96:117] = lo[1::2]
    return out


def _make_in_maps(TF, SF, t_out, s_out):
    TFr = np.ascontiguousarray(TF.reshape(B, C, M), dtype=np.float32)
    SFr = np.ascontiguousarray(SF.reshape(B, C, M), dtype=np.float32)
    TOhl = _hl_pack(np.asarray(t_out, dtype=np.float32).reshape(B, CC, M2))
    SOhl = _hl_pack(np.asarray(s_out, dtype=np.float32).reshape(B, CC, M2))
    in_maps = []
    for i in range(N_CORES):
        sl = slice(BPC * i, BPC * (i + 1))
        in_maps.append(
            {
                "TF": np.ascontiguousarray(TFr[sl]),
                "SF": np.ascontiguousarray(SFr[sl]),
                "TOHL": np.ascontiguousarray(TOhl[i]),
                "SOHL": np.ascontiguousarray(SOhl[i]),
            }
        )
    return in_maps


def _assemble(results, s_out):
    EPS = 1e-12
    sa_rho = 0.0
    ic_num = 0.0
    for r in results:
        st = r["OST"][0].astype(np.float64)
        ns2 = r["OST"][1].astype(np.float64)
        nt2 = r["OST"][2].astype(np.float64)
        sa_rho += float((st / np.maximum(np.sqrt(ns2 * nt2), EPS)).sum())
        gpair = []
        for ti in range(2):  # t, s
            gsb = r["OG"][ti].astype(np.float64)
            gb = []
            for bi, (hh, hl, ll) in enumerate(
                (((0, 0), (0, 32), (32, 32)), ((64, 64), (64, 96), (96, 96)))
            ):
                Ghh = gsb[hh[0] : hh[0] + CC, hh[1] : hh[1] + CC]
                Ghl = gsb[hl[0] : hl[0] + CC, hl[1] : hl[1] + CC]
                Gll = gsb[ll[0] : ll[0] + CC, ll[1] : ll[1] + CC]
                gb.append(Ghh + Ghl + Ghl.T + Gll)
            gpair.append(gb)
        for bi in range(BPC):
            Gt, Gs = gpair[0][bi], gpair[1][bi]
            Gtn = Gt / np.maximum(
                np.linalg.norm(Gt, axis=1, keepdims=True), EPS
            )
            Gsn = Gs / np.maximum(
                np.linalg.norm(Gs, axis=1, keepdims=True), EPS
            )
            d = Gsn - Gtn
            ic_num += float((d * d).sum())
    sa_loss = (2.0 * B * M - 2.0 * sa_rho) / (B * M * M)
    ic_loss = ic_num / (CC * B)
    return (s_out, np.float32(ic_loss), np.float32(sa_loss))


def kernel(TF, SF, t_out, s_out, _trace=False):
    nc = _get_nc()
    in_maps = _make_in_maps(TF, SF, t_out, s_out)
    res = run_bass_kernel_spmd(nc, in_maps, core_ids=list(range(N_CORES)), trace=_trace)
    out = _assemble(res.results, s_out)
    if _trace:
        return out, res
    return out
